# revision 20
# baseline (speedup 1.0000x reference)
"""Trainium2 Bass kernel for nn_Attention_43963285242601.

GQA attention block: q/k/v projections + RoPE + causal attention + o_proj,
tensor-parallel over 8 NeuronCores.

Sharding (core c of 8):
  - q-heads 4c..4c+3 and kv-head c: Wq/Wk/Wv column (head) shards,
    attention fully local per head group.
  - o_proj sharded over Wo ROWS (output features): every core computes
    out[:, 512c:512c+512] and needs the full attention output, distributed
    via one AllGather per batch (bf16).  The host concatenates the 8
    feature shards: no all-reduce needed.

v4 structure: one fused emission stream.  Projections for each 512-token
group run in two 256-token half-sweeps so all six projection accumulators
(k, v, q0..q3) fit in THREE PSUM banks (two column-halves per bank; only
the first matmul into a bank uses start=True, later blocks rely on the
per-element has_written bit).  Attention for a token group is emitted as
micro-units (score pair / softmax+PV pair / tail) interleaved between
later projection matmuls, so softmax (ACT) latency hides under PE work.
The causal mask is a post-exp bf16 multiply on DVE; softmax P/acc run in
bf16; the denominator sum/broadcast matmuls borrow score PSUM slots; the
PV accumulator is evacuated to SBUF immediately so its bank frees early.
The batch-0 AllGather fires mid-kernel; o_proj(b0) hides the batch-1
AllGather.
"""

import numpy as np

import concourse.bacc as bacc
import concourse.mybir as mybir
import concourse.tile as tile
from concourse.bass_utils import run_bass_kernel_spmd

F32 = mybir.dt.float32
F32R = mybir.dt.float32r
BF16 = mybir.dt.bfloat16
AF = mybir.ActivationFunctionType

N_CORES = 8
B, L = 2, 2048
N_HEADS, N_KV = 32, 8
HEAD_DIM = 128
D = N_HEADS * HEAD_DIM
THETA = 500000.0

EXP_BIAS = -8.0


def _rope_tables(t_all, l, dh):
    half = dh // 2
    inv = 1.0 / (THETA ** (np.arange(half, dtype=np.float64) * 2.0 / dh))
    pos = np.arange(t_all, dtype=np.float64) % l
    ang = inv[:, None] * pos[None, :]  # [half, T]
    cos = np.cos(ang)
    sin = np.sin(ang)
    return (
        np.concatenate([cos, cos], 0).astype(np.float32),
        np.concatenate([sin, sin], 0).astype(np.float32),
    )


def _build(n_cores=N_CORES, b=B, l=L, nh=N_HEADS, nkv=N_KV):
    import ml_dtypes

    dh = HEAD_DIM
    d = nh * dh
    t_all = b * l
    hpc = nh // n_cores  # q heads per core
    assert nkv == n_cores, "one kv head per core"
    mpc = d // n_cores  # o_proj output features per core
    kt_d = d // dh  # contraction tiles for projections
    tg_n = t_all // 512  # 512-wide token groups
    ksub = 4  # k-tiles per x subslab load
    nsub = kt_d // ksub
    scale = dh ** -0.5

    nc = bacc.Bacc(
        "TRN2", target_bir_lowering=False, debug=False, num_devices=n_cores
    )

    xT = nc.dram_tensor("xT", [d, t_all], F32R, kind="ExternalInput").ap()
    wqT = nc.dram_tensor("wqT", [d, hpc * dh], F32R, kind="ExternalInput").ap()
    wkT = nc.dram_tensor("wkT", [d, dh], F32R, kind="ExternalInput").ap()
    wvT = nc.dram_tensor("wvT", [d, dh], F32R, kind="ExternalInput").ap()
    woT = nc.dram_tensor("woT", [d, mpc], BF16, kind="ExternalInput").ap()
    outT = nc.dram_tensor("outT", [mpc, t_all], F32, kind="ExternalOutput").ap()

    # compile-time constants
    cos_np, sin_np = _rope_tables(t_all, l, dh)
    # multiplicative bf16 causal mask for the 4 diagonal key tiles of each
    # 512-query group: maskP[k, j*512 + q] = (128*j + k <= q)
    k_idx = np.arange(128)[:, None]
    q_idx = np.arange(512)[None, :]
    maskp_np = np.zeros((128, 4 * 512), dtype=ml_dtypes.bfloat16)
    for j in range(4):
        maskp_np[:, j * 512 : (j + 1) * 512] = (128 * j + k_idx <= q_idx).astype(
            ml_dtypes.bfloat16
        )
    cos_c = nc.inline_tensor(cos_np, name="cos_c").ap()
    sin_c = nc.inline_tensor(sin_np, name="sin_c").ap()
    maskp_c = nc.inline_tensor(maskp_np, name="maskp_c").ap()
    ident_c = nc.inline_tensor(np.eye(128, dtype=np.float32), name="ident_c").ap()
    ones_kb_c = nc.inline_tensor(
        np.ones((128, 1), dtype=ml_dtypes.bfloat16), name="ones_kb_c"
    ).ap()
    ones_rf_c = nc.inline_tensor(
        np.ones((1, 128), dtype=np.float32), name="ones_rf_c"
    ).ap()

    with tile.TileContext(nc) as tc:
        with (
            tc.tile_pool(name="constp", bufs=1) as constp,
            tc.tile_pool(name="kvp", bufs=1) as kvp,
            tc.tile_pool(name="qstp", bufs=8) as qstp,
            tc.tile_pool(name="cspool", bufs=2) as cspool,
            tc.tile_pool(name="ropet", bufs=3) as ropet,
            tc.tile_pool(name="vtst", bufs=2) as vtst,
            tc.tile_pool(name="ppool", bufs=3) as ppool,
            tc.tile_pool(name="accp", bufs=2) as accp,
            tc.tile_pool(name="rsb", bufs=2) as rsb,
            tc.tile_pool(name="obf", bufs=2) as obf,
            tc.tile_pool(name="dramp", bufs=1, space="DRAM") as dramp,
            tc.tile_pool(name="pspp", bufs=2, space="PSUM") as pspp,
            tc.tile_pool(name="pop", bufs=1, space="PSUM") as pop,
        ):
            masksb = constp.tile([128, 4 * 512], BF16, tag="masksb")
            nc.gpsimd.dma_start(masksb[:], maskp_c)
            ident = constp.tile([128, 128], F32, tag="ident")
            nc.gpsimd.dma_start(ident[:], ident_c)
            ones_kb = constp.tile([128, 1], BF16, tag="ones_kb")
            nc.gpsimd.dma_start(ones_kb[:], ones_kb_c)
            ones_rf = constp.tile([1, 128], F32, tag="ones_rf")
            nc.gpsimd.dma_start(ones_rf[:], ones_rf_c)
            bias_t = constp.tile([128, 1], F32, tag="bias_t")
            nc.vector.memset(bias_t[:], EXP_BIAS)

            # per-batch K (rotated, [dh, l]) and Vn ([key, dh-cols]) buffers
            K2 = [kvp.tile([128, l], F32R, tag=f"K{i}", name=f"K{i}") for i in range(2)]
            Vn2 = [
                kvp.tile([128, l], BF16, tag=f"Vn{i}", name=f"Vn{i}")
                for i in range(2)
            ]

            # one bounce/gather pair per batch: rows = local heads * dh
            bounce = [
                dramp.tile([hpc * dh, l], BF16, tag=f"bounce{bb}",
                           name=f"bounce{bb}")
                for bb in range(b)
            ]
            gathered = [
                dramp.tile(
                    [n_cores * hpc * dh, l], BF16,
                    addr_space="Shared" if n_cores > 4 else "Local",
                    tag=f"gath{bb}", name=f"gath{bb}"
                )
                for bb in range(b)
            ]

            wq_r = wqT.rearrange("(k p) m -> p k m", p=128)
            wk_r = wkT.rearrange("(k p) m -> p k m", p=128)
            wv_r = wvT.rearrange("(k p) m -> p k m", p=128)
            xT_r = xT.rearrange("(k p) t -> p k t", p=128)

            # ---- attention micro-unit machinery --------------------------
            # pending: FIFO of (batch, closure).  Units pop between
            # projection matmul blocks so softmax latency hides under PE
            # work that does not depend on it.
            pending = []

            def drain(k):
                for _ in range(min(k, len(pending))):
                    pending.pop(0)[1]()

            def drain_all():
                while pending:
                    pending.pop(0)[1]()

            def drain_batch(bb):
                while pending and pending[0][0] == bb:
                    pending.pop(0)[1]()

            def make_group_units(bb, h, g, qt):
                """Attention for (batch bb, local head h, 512-query group g).
                qt: rotated q tile [128, 512] f32r.  Appends units to pending."""
                nkt = 4 * g + 4
                npair = nkt // 2
                K = K2[bb]
                Vn = Vn2[bb]
                po = {}
                acc = {}
                state = {}

                def mk_scores(pr):
                    def u():
                        psp = pspp.tile([128, 1024], F32, tag="psp", name="psp")
                        state[("psp", pr)] = psp
                        for half in range(2):
                            kt = 2 * pr + half
                            nc.tensor.matmul(
                                psp[:, half * 512 : (half + 1) * 512],
                                K[:, kt * 128 : (kt + 1) * 128],
                                qt[:],
                                start=True,
                                stop=True,
                                skip_group_check=True,
                            )
                    return u

                def mk_softpv(pr):
                    def u():
                        psp = state.pop(("psp", pr))
                        P = ppool.tile([128, 1024], BF16, tag="P", name="P")
                        nc.scalar.activation(
                            P[:], psp[:], AF.Exp, scale=scale, bias=bias_t[:]
                        )
                        j0 = 2 * pr - 4 * g  # index into diagonal-mask blocks
                        if j0 >= 0:
                            nc.vector.tensor_mul(
                                P[:], P[:], masksb[:, j0 * 512 : (j0 + 2) * 512]
                            )
                        if pr == 0:
                            po["t"] = pop.tile([128, 512], F32, tag="po",
                                               name="po")
                            acc["t"] = accp.tile([128, 512], BF16, tag="acc",
                                                 name="acc")
                        for half in range(2):
                            kt = 2 * pr + half
                            Ph = P[:, half * 512 : (half + 1) * 512]
                            nc.tensor.matmul(
                                po["t"][:],
                                Vn[:, kt * 128 : (kt + 1) * 128],
                                Ph,
                                start=(kt == 0),
                                stop=(kt == nkt - 1),
                                skip_group_check=True,
                            )
                            if kt == 0:
                                nc.vector.tensor_copy(acc["t"][:], Ph)
                            else:
                                nc.vector.tensor_add(acc["t"][:], acc["t"][:], Ph)
                        if pr == npair - 1:
                            # evacuate po so its bank frees before the
                            # softmax tail; the tail runs off SBUF
                            po["sb"] = obf.tile([128, 512], F32, tag="posb",
                                                name="posb")
                            nc.scalar.activation(po["sb"][:], po["t"][:],
                                                 AF.Copy)
                    return u

                def tail():
                    # denominator: sum over keys via ones-matmul, broadcast
                    # via rank-1 matmul; both borrow score PSUM slots
                    pd_t = pspp.tile([1, 512], F32, tag="psp", name="pd")
                    nc.tensor.matmul(
                        pd_t[:], ones_kb[:], acc["t"][:],
                        start=True, stop=True, skip_group_check=True,
                    )
                    r_sb = rsb.tile([1, 512], F32, tag="r", name="r")
                    nc.vector.tensor_copy(r_sb[:], pd_t[:])
                    pb_t = pspp.tile([128, 512], F32, tag="psp", name="pb")
                    nc.tensor.matmul(
                        pb_t[:], ones_rf[:], r_sb[:],
                        start=True, stop=True, skip_group_check=True,
                    )
                    bs = rsb.tile([128, 512], F32, tag="bs", name="bs")
                    nc.vector.reciprocal(bs[:], pb_t[:])
                    ob = obf.tile([128, 512], BF16, tag="ob", name="ob")
                    nc.vector.tensor_mul(ob[:], po["sb"][:], bs[:])
                    nc.sync.dma_start(
                        bounce[bb][h * dh : (h + 1) * dh,
                                   g * 512 : (g + 1) * 512],
                        ob[:],
                    )

                # scores run one pair ahead of softmax+PV so exp latency is
                # always covered by in-flight PE work
                units = [mk_scores(0)]
                for pr in range(1, npair):
                    units.append(mk_scores(pr))
                    units.append(mk_softpv(pr - 1))
                units.append(mk_softpv(npair - 1))
                units.append(tail)
                pending.extend((bb, u) for u in units)

            # ---- fused projection + attention region ---------------------
            with (
                tc.tile_pool(name="wpool", bufs=1) as wpool,
                tc.tile_pool(name="xpool", bufs=3) as xpool,
                tc.tile_pool(name="psproj", bufs=1, space="PSUM") as psproj,
            ):
                wq_sb = wpool.tile([128, kt_d, hpc * dh], F32R, tag="wq")
                wk_sb = wpool.tile([128, kt_d, dh], F32R, tag="wk")
                wv_sb = wpool.tile([128, kt_d, dh], F32R, tag="wv")

                def emit_tg(tg):
                    bb = tg // 4
                    g = tg % 4
                    toff = tg * 512
                    tloc = g * 512

                    cos_sb = cspool.tile([128, 512], F32, tag="cos")
                    nc.gpsimd.dma_start(cos_sb[:], cos_c[:, toff : toff + 512])
                    sin_sb = cspool.tile([128, 512], F32, tag="sin")
                    nc.gpsimd.dma_start(sin_sb[:], sin_c[:, toff : toff + 512])

                    def rope(dst_ap, src_ap, c0):
                        # dst[0:64]  = s[0:64]*cos - s[64:]*sin
                        # dst[64:]   = s[64:]*cos + s[0:64]*sin
                        # 256-wide half; c0 = column offset into cos/sin slab
                        cs = cos_sb[:, c0 : c0 + 256]
                        tc_ = ropet.tile([128, 256], F32, tag="rtc", name="rtc")
                        ts_ = ropet.tile([128, 256], F32, tag="rts", name="rts")
                        nc.vector.tensor_mul(tc_[:], src_ap, cs)
                        nc.vector.tensor_mul(
                            ts_[0:64, :], src_ap[64:128, :],
                            sin_sb[64:128, c0 : c0 + 256],
                        )
                        nc.vector.tensor_mul(
                            ts_[64:128, :], src_ap[0:64, :],
                            sin_sb[0:64, c0 : c0 + 256],
                        )
                        nc.vector.tensor_sub(
                            dst_ap[0:64, :], tc_[0:64, :], ts_[0:64, :]
                        )
                        nc.vector.tensor_add(
                            dst_ap[64:128, :], tc_[64:128, :], ts_[64:128, :]
                        )

                    qts = [qstp.tile([128, 512], F32R, tag="qst", name="qst")
                           for _ in range(hpc)]
                    vt = vtst.tile([128, 512], F32, tag="vt", name="vt")

                    # two 256-token half-sweeps over x; six projection
                    # accumulators live in three PSUM banks (column halves,
                    # has_written-based accumulation: only the first matmul
                    # into each bank sets start=True)
                    for half in range(2):
                        th = toff + half * 256
                        pkv = psproj.tile([128, 512], F32, tag="ga", name="ga")
                        pq01 = psproj.tile([128, 512], F32, tag="gb", name="gb")
                        pq23 = psproj.tile([128, 512], F32, tag="gc", name="gc")
                        views = [
                            (pkv, 0, wk_sb, 0),
                            (pkv, 1, wv_sb, 0),
                            (pq01, 0, wq_sb, 0),
                            (pq01, 1, wq_sb, dh),
                            (pq23, 0, wq_sb, 2 * dh),
                            (pq23, 1, wq_sb, 3 * dh),
                        ]
                        for sub in range(nsub):
                            ks = slice(sub * ksub, (sub + 1) * ksub)
                            if tg == 0 and half == 0:
                                nc.gpsimd.dma_start(wq_sb[:, ks, :], wq_r[:, ks, :])
                                nc.gpsimd.dma_start(wk_sb[:, ks, :], wk_r[:, ks, :])
                                nc.gpsimd.dma_start(wv_sb[:, ks, :], wv_r[:, ks, :])
                            xs = xpool.tile([128, ksub, 256], F32R, tag="xs")
                            nc.sync.dma_start(xs[:], xT_r[:, ks, th : th + 256])
                            for ptile, col, w_sb, o0 in views:
                                csl = slice(col * 256, (col + 1) * 256)
                                for k in range(ksub):
                                    kt = sub * ksub + k
                                    nc.tensor.matmul(
                                        ptile[:, csl],
                                        w_sb[:, kt, o0 : o0 + dh],
                                        xs[:, k, :],
                                        start=(kt == 0 and col == 0),
                                        stop=(kt == kt_d - 1),
                                        skip_group_check=True,
                                    )
                                drain(1)

                        # drain this half's accumulators
                        c0 = half * 256
                        rope(K2[bb][:, tloc + c0 : tloc + c0 + 256],
                             pkv[:, 0:256], c0)
                        nc.scalar.activation(
                            vt[:, c0 : c0 + 256], pkv[:, 256:512], AF.Copy
                        )
                        rope(qts[0][:, c0 : c0 + 256], pq01[:, 0:256], c0)
                        rope(qts[1][:, c0 : c0 + 256], pq01[:, 256:512], c0)
                        rope(qts[2][:, c0 : c0 + 256], pq23[:, 0:256], c0)
                        rope(qts[3][:, c0 : c0 + 256], pq23[:, 256:512], c0)
                        drain(2)

                    # v: transpose 128-blocks on PE into Vn
                    pt = pspp.tile([128, 512], F32, tag="psp", name="pt")
                    for j in range(4):
                        nc.tensor.transpose(
                            pt[:, j * 128 : (j + 1) * 128],
                            vt[:, j * 128 : (j + 1) * 128],
                            ident[:],
                        )
                    nc.vector.tensor_copy(Vn2[bb][:, tloc : tloc + 512], pt[:])

                    # enqueue this token group's attention
                    for h in range(hpc):
                        make_group_units(bb, h, g, qts[h])
                        drain(2)

                for tg in range(tg_n):
                    emit_tg(tg)
                    if tg == 5:
                        # batch-0 groups are all enqueued by tg3; force
                        # stragglers out so bounce writes precede the
                        # collective in emission order
                        drain_batch(0)
                        nc.gpsimd.collective_compute(
                            "AllGather",
                            mybir.AluOpType.bypass,
                            replica_groups=[list(range(n_cores))],
                            ins=[bounce[0].opt()],
                            outs=[gathered[0].opt()],
                        )

            # wq/wk/wv/xs pools and proj PSUM released here.
            with (
                tc.tile_pool(name="wopool", bufs=1) as wopool,
                tc.tile_pool(name="ogpool", bufs=2) as ogpool,
                tc.tile_pool(name="outst", bufs=3) as outst,
                tc.tile_pool(name="pso", bufs=2, space="PSUM") as pso,
            ):
                wo_sb = wopool.tile([128, kt_d, mpc], BF16, tag="wo")
                nc.gpsimd.dma_start(
                    wo_sb[:], woT.rearrange("(k p) m -> p k m", p=128)
                )

                def oproj_slab(bb, tgl):
                    # gathered rows: rank r block holds global heads
                    # 4r..4r+3, so k-tile index == Wo column block index
                    g_r = gathered[bb][:].rearrange("(k p) t -> p k t", p=128)
                    og = ogpool.tile([128, kt_d, 512], BF16, tag="og", name="og")
                    nc.gpsimd.dma_start(
                        og[:], g_r[:, :, tgl * 512 : (tgl + 1) * 512]
                    )
                    # keep leftover attention units ahead of matmuls that
                    # wait on the og/wo DMAs
                    drain(2)
                    for m in range(mpc // 128):
                        pp = pso.tile([128, 512], F32, tag="pp", name="pp")
                        for kt in range(kt_d):
                            nc.tensor.matmul(
                                pp[:],
                                wo_sb[:, kt, m * 128 : (m + 1) * 128],
                                og[:, kt, :],
                                start=(kt == 0),
                                stop=(kt == kt_d - 1),
                            )
                            if kt % 4 == 3:
                                drain(1)
                        ot = outst.tile([128, 512], F32, tag="ot", name="ot")
                        nc.scalar.activation(ot[:], pp[:], AF.Copy)
                        nc.sync.dma_start(
                            outT[
                                m * 128 : (m + 1) * 128,
                                bb * l + tgl * 512 : bb * l + (tgl + 1) * 512,
                            ],
                            ot[:],
                        )

                # batch-0 o_proj starts immediately (its gather landed mid-
                # kernel); leftover batch-1 attention drains under it, then
                # the batch-1 AllGather fires and hides under the rest
                drain(6)
                oproj_slab(0, 0)
                oproj_slab(0, 1)
                drain_all()
                nc.gpsimd.collective_compute(
                    "AllGather",
                    mybir.AluOpType.bypass,
                    replica_groups=[list(range(n_cores))],
                    ins=[bounce[1].opt()],
                    outs=[gathered[1].opt()],
                )
                oproj_slab(0, 2)
                oproj_slab(0, 3)
                for tgl in range(l // 512):
                    oproj_slab(1, tgl)

    nc.compile()
    return nc


_NC_CACHE = {}


def _get_nc(key=(N_CORES, B, L, N_HEADS, N_KV)):
    if key not in _NC_CACHE:
        _NC_CACHE[key] = _build(*key)
    return _NC_CACHE[key]


def make_in_maps(x, Wq, Wk, Wv, Wo, n_cores=N_CORES):
    import ml_dtypes

    b, l, d = x.shape
    nh = Wq.shape[0] // HEAD_DIM
    hpc = nh // n_cores
    mpc = d // n_cores
    xT = np.ascontiguousarray(x.reshape(b * l, d).T.astype(np.float32))
    in_maps = []
    for c in range(n_cores):
        wq_c = np.ascontiguousarray(
            Wq[c * hpc * HEAD_DIM : (c + 1) * hpc * HEAD_DIM, :].T.astype(np.float32)
        )
        wk_c = np.ascontiguousarray(
            Wk[c * HEAD_DIM : (c + 1) * HEAD_DIM, :].T.astype(np.float32)
        )
        wv_c = np.ascontiguousarray(
            Wv[c * HEAD_DIM : (c + 1) * HEAD_DIM, :].T.astype(np.float32)
        )
        wo_c = np.ascontiguousarray(
            Wo[c * mpc : (c + 1) * mpc, :].T.astype(ml_dtypes.bfloat16)
        )
        in_maps.append(
            {"xT": xT, "wqT": wq_c, "wkT": wk_c, "wvT": wv_c, "woT": wo_c}
        )
    return in_maps


def assemble_out(results, b, l, d):
    parts = [r["outT"] for r in results]
    outT = np.concatenate(parts, axis=0)  # [D, T]
    return np.ascontiguousarray(outT.T).reshape(b, l, d).astype(np.float32)


def kernel(x, Wq, Wk, Wv, Wo, trace=False, tmpdir=None):
    x = np.asarray(x, dtype=np.float32)
    nc = _get_nc()
    in_maps = make_in_maps(x, Wq, Wk, Wv, Wo)
    res = run_bass_kernel_spmd(
        nc, in_maps, list(range(N_CORES)), trace=trace, tmpdir=tmpdir
    )
    out = assemble_out(res.results, *x.shape)
    if trace:
        return out, res
    return out


if __name__ == "__main__":
    rng = np.random.default_rng(0)
    s = 0.02
    x = rng.standard_normal((B, L, D)).astype(np.float32)
    Wq = (rng.standard_normal((D, D)) * s).astype(np.float32)
    Wk = (rng.standard_normal((N_KV * HEAD_DIM, D)) * s).astype(np.float32)
    Wv = (rng.standard_normal((N_KV * HEAD_DIM, D)) * s).astype(np.float32)
    Wo = (rng.standard_normal((D, D)) * s).astype(np.float32)
    out = kernel(x, Wq, Wk, Wv, Wo)
    print(out.shape, out.dtype)


# revision 22
# speedup vs baseline: 1.0025x; 1.0025x over previous
"""Trainium2 Bass kernel for nn_Attention_43963285242601.

GQA attention block: q/k/v projections + RoPE + causal attention + o_proj,
tensor-parallel over 8 NeuronCores.

Sharding (core c of 8):
  - q-heads 4c..4c+3 and kv-head c: Wq/Wk/Wv column (head) shards,
    attention fully local per head group.
  - o_proj sharded over Wo ROWS (output features): every core computes
    out[:, 512c:512c+512] and needs the full attention output, distributed
    via one AllGather per batch (bf16).  The host concatenates the 8
    feature shards: no all-reduce needed.

v5: per-batch phasing  proj(b0) -> attn(b0) -> AG(b0) -> proj(b1) ->
attn(b1) -> AG(b1) -> o_proj(b0) -> o_proj(b1), so each AllGather hides
under the following compute phase.  q stays in SBUF (no DRAM spill),
RoPE runs on DVE straight out of projection PSUM, softmax P/acc are bf16,
the causal mask is a post-exp bf16 multiply on DVE (no mask matmuls), the
PV accumulator is evacuated to SBUF immediately so its PSUM bank frees
before the softmax tail, and o_proj uses an identity head mapping thanks
to the combined per-batch gather layout.
"""

import numpy as np

import concourse.bacc as bacc
import concourse.mybir as mybir
import concourse.tile as tile
from concourse.bass_utils import run_bass_kernel_spmd

F32 = mybir.dt.float32
F32R = mybir.dt.float32r
BF16 = mybir.dt.bfloat16
AF = mybir.ActivationFunctionType

N_CORES = 8
B, L = 2, 2048
N_HEADS, N_KV = 32, 8
HEAD_DIM = 128
D = N_HEADS * HEAD_DIM
THETA = 500000.0

EXP_BIAS = -8.0


def _rope_tables(t_all, l, dh):
    half = dh // 2
    inv = 1.0 / (THETA ** (np.arange(half, dtype=np.float64) * 2.0 / dh))
    pos = np.arange(t_all, dtype=np.float64) % l
    ang = inv[:, None] * pos[None, :]  # [half, T]
    cos = np.cos(ang)
    sin = np.sin(ang)
    return (
        np.concatenate([cos, cos], 0).astype(np.float32),
        np.concatenate([sin, sin], 0).astype(np.float32),
    )


def _build(n_cores=N_CORES, b=B, l=L, nh=N_HEADS, nkv=N_KV):
    import ml_dtypes

    dh = HEAD_DIM
    d = nh * dh
    t_all = b * l
    hpc = nh // n_cores  # q heads per core
    assert nkv == n_cores, "one kv head per core"
    mpc = d // n_cores  # o_proj output features per core
    kt_d = d // dh  # contraction tiles for projections
    qg_n = l // 512  # 512-wide query groups per batch
    ksub = 4  # k-tiles per x subslab load
    nsub = kt_d // ksub
    scale = dh ** -0.5

    nc = bacc.Bacc(
        "TRN2", target_bir_lowering=False, debug=False, num_devices=n_cores
    )

    xT = nc.dram_tensor("xT", [d, t_all], F32R, kind="ExternalInput").ap()
    wqT = nc.dram_tensor("wqT", [d, hpc * dh], F32R, kind="ExternalInput").ap()
    wkT = nc.dram_tensor("wkT", [d, dh], F32R, kind="ExternalInput").ap()
    wvT = nc.dram_tensor("wvT", [d, dh], F32R, kind="ExternalInput").ap()
    woT = nc.dram_tensor("woT", [d, mpc], BF16, kind="ExternalInput").ap()
    outT = nc.dram_tensor("outT", [mpc, t_all], F32, kind="ExternalOutput").ap()

    # compile-time constants
    cos_np, sin_np = _rope_tables(t_all, l, dh)
    # multiplicative bf16 causal mask for the 4 diagonal key tiles of each
    # 512-query group: maskP[k, j*512 + q] = (128*j + k <= q)
    k_idx = np.arange(128)[:, None]
    q_idx = np.arange(512)[None, :]
    maskp_np = np.zeros((128, 4 * 512), dtype=ml_dtypes.bfloat16)
    for j in range(4):
        maskp_np[:, j * 512 : (j + 1) * 512] = (128 * j + k_idx <= q_idx).astype(
            ml_dtypes.bfloat16
        )
    cos_c = nc.inline_tensor(cos_np, name="cos_c").ap()
    sin_c = nc.inline_tensor(sin_np, name="sin_c").ap()
    maskp_c = nc.inline_tensor(maskp_np, name="maskp_c").ap()
    ident_c = nc.inline_tensor(np.eye(128, dtype=np.float32), name="ident_c").ap()
    ones_kb_c = nc.inline_tensor(
        np.ones((128, 1), dtype=ml_dtypes.bfloat16), name="ones_kb_c"
    ).ap()
    ones_rf_c = nc.inline_tensor(
        np.ones((1, 128), dtype=np.float32), name="ones_rf_c"
    ).ap()

    with tile.TileContext(nc) as tc:
        with (
            tc.tile_pool(name="constp", bufs=1) as constp,
            tc.tile_pool(name="kvp", bufs=1) as kvp,
            tc.tile_pool(name="qpool", bufs=1) as qpool,
            tc.tile_pool(name="cspool", bufs=2) as cspool,
            tc.tile_pool(name="ropet", bufs=2) as ropet,
            tc.tile_pool(name="vtst", bufs=1) as vtst,
            tc.tile_pool(name="ppool", bufs=2) as ppool,
            tc.tile_pool(name="accp", bufs=2) as accp,
            tc.tile_pool(name="rsb", bufs=1) as rsb,
            tc.tile_pool(name="obf", bufs=2) as obf,
            tc.tile_pool(name="dramp", bufs=1, space="DRAM") as dramp,
        ):
            masksb = constp.tile([128, 4 * 512], BF16, tag="masksb")
            nc.gpsimd.dma_start(masksb[:], maskp_c)
            ident = constp.tile([128, 128], F32, tag="ident")
            nc.gpsimd.dma_start(ident[:], ident_c)
            ones_kb = constp.tile([128, 1], BF16, tag="ones_kb")
            nc.gpsimd.dma_start(ones_kb[:], ones_kb_c)
            ones_rf = constp.tile([1, 128], F32, tag="ones_rf")
            nc.gpsimd.dma_start(ones_rf[:], ones_rf_c)
            bias_t = constp.tile([128, 1], F32, tag="bias_t")
            nc.vector.memset(bias_t[:], EXP_BIAS)

            # per-batch K (rotated, [dh, l]) and Vn ([key, dh-cols]) buffers
            K2 = [kvp.tile([128, l], F32R, tag=f"K{i}", name=f"K{i}") for i in range(2)]
            Vn2 = [
                kvp.tile([128, l], BF16, tag=f"Vn{i}", name=f"Vn{i}")
                for i in range(2)
            ]

            # one bounce/gather pair per batch: rows = local heads * dh
            bounce = [
                dramp.tile([hpc * dh, l], BF16, tag=f"bounce{bb}",
                           name=f"bounce{bb}")
                for bb in range(b)
            ]
            gathered = [
                dramp.tile(
                    [n_cores * hpc * dh, l], BF16,
                    addr_space="Shared" if n_cores > 4 else "Local",
                    tag=f"gath{bb}", name=f"gath{bb}"
                )
                for bb in range(b)
            ]

            wq_r = wqT.rearrange("(k p) m -> p k m", p=128)
            wk_r = wkT.rearrange("(k p) m -> p k m", p=128)
            wv_r = wvT.rearrange("(k p) m -> p k m", p=128)
            xT_r = xT.rearrange("(k p) t -> p k t", p=128)

            def emit_proj_batch(bb, psq, xpool, wq_sb, wk_sb, wv_sb, q_sb):
                """Projections + RoPE + V transpose for batch bb (4 tgs)."""
                for g in range(qg_n):
                    tg = bb * qg_n + g
                    toff = tg * 512
                    tloc = g * 512

                    cos_sb = cspool.tile([128, 512], F32, tag="cos")
                    nc.gpsimd.dma_start(cos_sb[:], cos_c[:, toff : toff + 512])
                    sin_sb = cspool.tile([128, 512], F32, tag="sin")
                    nc.gpsimd.dma_start(sin_sb[:], sin_c[:, toff : toff + 512])

                    pq = [
                        psq.tile([128, 512], F32, tag=f"pq{o}", name=f"pq{o}")
                        for o in range(hpc)
                    ]
                    pk = psq.tile([128, 512], F32, tag="pk")
                    pv = psq.tile([128, 512], F32, tag="pv")
                    for sub in range(nsub):
                        ks = slice(sub * ksub, (sub + 1) * ksub)
                        if tg == 0:
                            nc.gpsimd.dma_start(wq_sb[:, ks, :], wq_r[:, ks, :])
                            nc.gpsimd.dma_start(wk_sb[:, ks, :], wk_r[:, ks, :])
                            nc.gpsimd.dma_start(wv_sb[:, ks, :], wv_r[:, ks, :])
                        xs = xpool.tile([128, ksub, 512], F32R, tag="xs")
                        nc.sync.dma_start(xs[:], xT_r[:, ks, toff : toff + 512])
                        for o in range(hpc):
                            for k in range(ksub):
                                kt = sub * ksub + k
                                nc.tensor.matmul(
                                    pq[o][:],
                                    wq_sb[:, kt, o * dh : (o + 1) * dh],
                                    xs[:, k, :],
                                    start=(kt == 0),
                                    stop=(kt == kt_d - 1),
                                )
                        for k in range(ksub):
                            kt = sub * ksub + k
                            nc.tensor.matmul(
                                pk[:], wk_sb[:, kt, :], xs[:, k, :],
                                start=(kt == 0), stop=(kt == kt_d - 1),
                            )
                        for k in range(ksub):
                            kt = sub * ksub + k
                            nc.tensor.matmul(
                                pv[:], wv_sb[:, kt, :], xs[:, k, :],
                                start=(kt == 0), stop=(kt == kt_d - 1),
                            )

                    def rope(dst_ap, src_ap):
                        # dst[0:64]  = s[0:64]*cos - s[64:]*sin
                        # dst[64:]   = s[64:]*cos + s[0:64]*sin
                        tc_ = ropet.tile([128, 512], F32, tag="rtc", name="rtc")
                        ts_ = ropet.tile([128, 512], F32, tag="rts", name="rts")
                        nc.vector.tensor_mul(tc_[:], src_ap, cos_sb[:])
                        nc.vector.tensor_mul(
                            ts_[0:64, :], src_ap[64:128, :], sin_sb[64:128, :]
                        )
                        nc.vector.tensor_mul(
                            ts_[64:128, :], src_ap[0:64, :], sin_sb[0:64, :]
                        )
                        nc.vector.tensor_sub(
                            dst_ap[0:64, :], tc_[0:64, :], ts_[0:64, :]
                        )
                        nc.vector.tensor_add(
                            dst_ap[64:128, :], tc_[64:128, :], ts_[64:128, :]
                        )

                    # ropes straight from PSUM (DVE), K into K2, q into SBUF
                    rope(K2[bb][:, tloc : tloc + 512], pk[:])
                    for o in range(hpc):
                        rope(q_sb[o][:, tloc : tloc + 512], pq[o][:])
                    # v: ACT copy + PE transpose into Vn
                    vt = vtst.tile([128, 512], F32, tag="vt", name="vt")
                    nc.scalar.activation(vt[:], pv[:], AF.Copy)
                    pt = psq.tile([128, 512], F32, tag="pm", name="pm")
                    for j in range(4):
                        nc.tensor.transpose(
                            pt[:, j * 128 : (j + 1) * 128],
                            vt[:, j * 128 : (j + 1) * 128],
                            ident[:],
                        )
                    nc.vector.tensor_copy(Vn2[bb][:, tloc : tloc + 512], pt[:])

            def emit_attn_group(bb, h, g, q_sb, pspp, pop, pdp):
                nkt = 4 * g + 4
                npair = nkt // 2
                K = K2[bb]
                Vn = Vn2[bb]
                qt = q_sb[h][:, g * 512 : (g + 1) * 512]
                po = pop.tile([128, 512], F32, tag="po", name="po")
                acc = accp.tile([128, 512], BF16, tag="acc", name="acc")
                for pr in range(npair):
                    psp = pspp.tile([128, 1024], F32, tag="psp", name="psp")
                    for half in range(2):
                        kt = 2 * pr + half
                        nc.tensor.matmul(
                            psp[:, half * 512 : (half + 1) * 512],
                            K[:, kt * 128 : (kt + 1) * 128],
                            qt,
                            start=True,
                            stop=True,
                            skip_group_check=True,
                        )
                    P = ppool.tile([128, 1024], BF16, tag="P", name="P")
                    nc.scalar.activation(
                        P[:], psp[:], AF.Exp, scale=scale, bias=bias_t[:]
                    )
                    j0 = 2 * pr - 4 * g
                    if j0 >= 0:
                        nc.vector.tensor_mul(
                            P[:], P[:], masksb[:, j0 * 512 : (j0 + 2) * 512]
                        )
                    for half in range(2):
                        kt = 2 * pr + half
                        Ph = P[:, half * 512 : (half + 1) * 512]
                        nc.tensor.matmul(
                            po[:],
                            Vn[:, kt * 128 : (kt + 1) * 128],
                            Ph,
                            start=(kt == 0),
                            stop=(kt == nkt - 1),
                            skip_group_check=True,
                        )
                        if kt == 0:
                            nc.vector.tensor_copy(acc[:], Ph)
                        else:
                            nc.vector.tensor_add(acc[:], acc[:], Ph)
                # evacuate po so its bank frees before the softmax tail
                po_sb = obf.tile([128, 512], F32, tag="posb", name="posb")
                nc.scalar.activation(po_sb[:], po[:], AF.Copy)
                # denominator (sum over keys) + broadcast via matmuls
                pd_t = pdp.tile([1, 512], F32, tag="pd", name="pd")
                nc.tensor.matmul(
                    pd_t[:], ones_kb[:], acc[:],
                    start=True, stop=True, skip_group_check=True,
                )
                r_sb = rsb.tile([1, 512], F32, tag="r", name="r")
                nc.vector.tensor_copy(r_sb[:], pd_t[:])
                pb_t = pdp.tile([128, 512], F32, tag="pd", name="pb")
                nc.tensor.matmul(
                    pb_t[:], ones_rf[:], r_sb[:],
                    start=True, stop=True, skip_group_check=True,
                )
                bs = rsb.tile([128, 512], F32, tag="bs", name="bs")
                nc.vector.reciprocal(bs[:], pb_t[:])
                ob = obf.tile([128, 512], BF16, tag="ob", name="ob")
                nc.vector.tensor_mul(ob[:], po_sb[:], bs[:])
                nc.sync.dma_start(
                    bounce[bb][h * dh : (h + 1) * dh, g * 512 : (g + 1) * 512],
                    ob[:],
                )

            def emit_ag(bb):
                nc.gpsimd.collective_compute(
                    "AllGather",
                    mybir.AluOpType.bypass,
                    replica_groups=[list(range(n_cores))],
                    ins=[bounce[bb].opt()],
                    outs=[gathered[bb].opt()],
                )

            # ---------------- per-batch pipeline --------------------------
            with (
                tc.tile_pool(name="wpool", bufs=1) as wpool,
                tc.tile_pool(name="xpool", bufs=2) as xpool,
            ):
                wq_sb = wpool.tile([128, kt_d, hpc * dh], F32R, tag="wq")
                wk_sb = wpool.tile([128, kt_d, dh], F32R, tag="wk")
                wv_sb = wpool.tile([128, kt_d, dh], F32R, tag="wv")

                for bb in range(b):
                    q_sb = [
                        qpool.tile([128, l], F32R, tag=f"qh{o}", name=f"qh{o}")
                        for o in range(hpc)
                    ]
                    with tc.tile_pool(name="psq", bufs=1, space="PSUM") as psq:
                        emit_proj_batch(bb, psq, xpool, wq_sb, wk_sb, wv_sb,
                                        q_sb)
                    with (
                        tc.tile_pool(name="pspp", bufs=2, space="PSUM") as pspp,
                        tc.tile_pool(name="pop", bufs=2, space="PSUM") as pop,
                        tc.tile_pool(name="pdp", bufs=1, space="PSUM") as pdp,
                    ):
                        for h in range(hpc):
                            for g in range(qg_n):
                                emit_attn_group(bb, h, g, q_sb, pspp, pop, pdp)
                    emit_ag(bb)

            # ---------------- o_proj --------------------------------------
            with (
                tc.tile_pool(name="wopool", bufs=1) as wopool,
                tc.tile_pool(name="ogpool", bufs=2) as ogpool,
                tc.tile_pool(name="outst", bufs=3) as outst,
                tc.tile_pool(name="pso", bufs=2, space="PSUM") as pso,
            ):
                wo_sb = wopool.tile([128, kt_d, mpc], BF16, tag="wo")
                nc.gpsimd.dma_start(
                    wo_sb[:], woT.rearrange("(k p) m -> p k m", p=128)
                )

                def oproj_slab(bb, tgl):
                    # gathered rows: rank r block holds global heads
                    # 4r..4r+3, so k-tile index == Wo column block index
                    g_r = gathered[bb][:].rearrange("(k p) t -> p k t", p=128)
                    og = ogpool.tile([128, kt_d, 512], BF16, tag="og", name="og")
                    nc.gpsimd.dma_start(
                        og[:], g_r[:, :, tgl * 512 : (tgl + 1) * 512]
                    )
                    for m in range(mpc // 128):
                        pp = pso.tile([128, 512], F32, tag="pp", name="pp")
                        for kt in range(kt_d):
                            nc.tensor.matmul(
                                pp[:],
                                wo_sb[:, kt, m * 128 : (m + 1) * 128],
                                og[:, kt, :],
                                start=(kt == 0),
                                stop=(kt == kt_d - 1),
                            )
                        ot = outst.tile([128, 512], F32, tag="ot", name="ot")
                        nc.scalar.activation(ot[:], pp[:], AF.Copy)
                        nc.sync.dma_start(
                            outT[
                                m * 128 : (m + 1) * 128,
                                bb * l + tgl * 512 : bb * l + (tgl + 1) * 512,
                            ],
                            ot[:],
                        )

                for bb in range(b):
                    for tgl in range(l // 512):
                        oproj_slab(bb, tgl)

    nc.compile()
    return nc


_NC_CACHE = {}


def _get_nc(key=(N_CORES, B, L, N_HEADS, N_KV)):
    if key not in _NC_CACHE:
        _NC_CACHE[key] = _build(*key)
    return _NC_CACHE[key]


def make_in_maps(x, Wq, Wk, Wv, Wo, n_cores=N_CORES):
    import ml_dtypes

    b, l, d = x.shape
    nh = Wq.shape[0] // HEAD_DIM
    hpc = nh // n_cores
    mpc = d // n_cores
    xT = np.ascontiguousarray(x.reshape(b * l, d).T.astype(np.float32))
    in_maps = []
    for c in range(n_cores):
        wq_c = np.ascontiguousarray(
            Wq[c * hpc * HEAD_DIM : (c + 1) * hpc * HEAD_DIM, :].T.astype(np.float32)
        )
        wk_c = np.ascontiguousarray(
            Wk[c * HEAD_DIM : (c + 1) * HEAD_DIM, :].T.astype(np.float32)
        )
        wv_c = np.ascontiguousarray(
            Wv[c * HEAD_DIM : (c + 1) * HEAD_DIM, :].T.astype(np.float32)
        )
        wo_c = np.ascontiguousarray(
            Wo[c * mpc : (c + 1) * mpc, :].T.astype(ml_dtypes.bfloat16)
        )
        in_maps.append(
            {"xT": xT, "wqT": wq_c, "wkT": wk_c, "wvT": wv_c, "woT": wo_c}
        )
    return in_maps


def assemble_out(results, b, l, d):
    parts = [r["outT"] for r in results]
    outT = np.concatenate(parts, axis=0)  # [D, T]
    return np.ascontiguousarray(outT.T).reshape(b, l, d).astype(np.float32)


def kernel(x, Wq, Wk, Wv, Wo, trace=False, tmpdir=None):
    x = np.asarray(x, dtype=np.float32)
    nc = _get_nc()
    in_maps = make_in_maps(x, Wq, Wk, Wv, Wo)
    res = run_bass_kernel_spmd(
        nc, in_maps, list(range(N_CORES)), trace=trace, tmpdir=tmpdir
    )
    out = assemble_out(res.results, *x.shape)
    if trace:
        return out, res
    return out


if __name__ == "__main__":
    rng = np.random.default_rng(0)
    s = 0.02
    x = rng.standard_normal((B, L, D)).astype(np.float32)
    Wq = (rng.standard_normal((D, D)) * s).astype(np.float32)
    Wk = (rng.standard_normal((N_KV * HEAD_DIM, D)) * s).astype(np.float32)
    Wv = (rng.standard_normal((N_KV * HEAD_DIM, D)) * s).astype(np.float32)
    Wo = (rng.standard_normal((D, D)) * s).astype(np.float32)
    out = kernel(x, Wq, Wk, Wv, Wo)
    print(out.shape, out.dtype)


# revision 23
# speedup vs baseline: 1.2620x; 1.2589x over previous
"""Trainium2 Bass kernel for nn_Attention_43963285242601.

GQA attention block: q/k/v projections + RoPE + causal attention + o_proj,
tensor-parallel over 8 NeuronCores.

Sharding (core c of 8):
  - q-heads 4c..4c+3 and kv-head c: Wq/Wk/Wv column (head) shards,
    attention fully local per head group.
  - o_proj sharded over Wo ROWS (output features): every core computes
    out[:, 512c:512c+512] and needs the full attention output, distributed
    via one AllGather per batch (bf16).  The host concatenates the 8
    feature shards: no all-reduce needed.

v6: fused emission stream.  Projections (six PSUM accumulators, sub-major
like the baseline) run per 512-token group; attention for earlier groups
is emitted as single-key-tile micro-units interleaved between projection
matmul blocks, so softmax (ACT) latency and all softmax-tail latencies
hide under PE work that does not depend on them.  K / q / scores / P run
in bf16 (bf16 LDWEIGHTS fully hides under the matmul stream; fp32
weights do not), the causal mask is a post-exp bf16 multiply on DVE, the
PV accumulator is evacuated to SBUF immediately (its PSUM bank frees
after one ACT copy), and the softmax denominator uses sum/broadcast
matmuls that borrow the score PSUM slot plus a fast-approximate
reciprocal (the exact DVE reciprocal costs 4us and serialized the DVE
queue).  One AllGather per batch: batch 0 fires mid-kernel, batch 1
hides under o_proj(b0); o_proj uses an identity head mapping thanks to
the combined gather layout.
"""

import numpy as np

import concourse.bacc as bacc
import concourse.mybir as mybir
import concourse.tile as tile
from concourse.bass_utils import run_bass_kernel_spmd

F32 = mybir.dt.float32
F32R = mybir.dt.float32r
BF16 = mybir.dt.bfloat16
AF = mybir.ActivationFunctionType

N_CORES = 8
B, L = 2, 2048
N_HEADS, N_KV = 32, 8
HEAD_DIM = 128
D = N_HEADS * HEAD_DIM
THETA = 500000.0

EXP_BIAS = -8.0


def _rope_tables(t_all, l, dh):
    half = dh // 2
    inv = 1.0 / (THETA ** (np.arange(half, dtype=np.float64) * 2.0 / dh))
    pos = np.arange(t_all, dtype=np.float64) % l
    ang = inv[:, None] * pos[None, :]  # [half, T]
    cos = np.cos(ang)
    sin = np.sin(ang)
    return (
        np.concatenate([cos, cos], 0).astype(np.float32),
        np.concatenate([sin, sin], 0).astype(np.float32),
    )


def _build(n_cores=N_CORES, b=B, l=L, nh=N_HEADS, nkv=N_KV):
    import ml_dtypes

    dh = HEAD_DIM
    d = nh * dh
    t_all = b * l
    hpc = nh // n_cores  # q heads per core
    assert nkv == n_cores, "one kv head per core"
    mpc = d // n_cores  # o_proj output features per core
    kt_d = d // dh  # contraction tiles for projections
    tg_n = t_all // 512  # 512-wide token groups
    ksub = 4  # k-tiles per x subslab load
    nsub = kt_d // ksub
    scale = dh ** -0.5

    nc = bacc.Bacc(
        "TRN2", target_bir_lowering=False, debug=False, num_devices=n_cores
    )

    xT = nc.dram_tensor("xT", [d, t_all], F32R, kind="ExternalInput").ap()
    wqT = nc.dram_tensor("wqT", [d, hpc * dh], F32R, kind="ExternalInput").ap()
    wkT = nc.dram_tensor("wkT", [d, dh], F32R, kind="ExternalInput").ap()
    wvT = nc.dram_tensor("wvT", [d, dh], F32R, kind="ExternalInput").ap()
    woT = nc.dram_tensor("woT", [d, mpc], BF16, kind="ExternalInput").ap()
    outT = nc.dram_tensor("outT", [mpc, t_all], F32, kind="ExternalOutput").ap()

    # compile-time constants
    cos_np, sin_np = _rope_tables(t_all, l, dh)
    # multiplicative bf16 causal mask for the 4 diagonal key tiles of each
    # 512-query group: maskP[k, j*512 + q] = (128*j + k <= q)
    k_idx = np.arange(128)[:, None]
    q_idx = np.arange(512)[None, :]
    maskp_np = np.zeros((128, 4 * 512), dtype=ml_dtypes.bfloat16)
    for j in range(4):
        maskp_np[:, j * 512 : (j + 1) * 512] = (128 * j + k_idx <= q_idx).astype(
            ml_dtypes.bfloat16
        )
    cos_c = nc.inline_tensor(cos_np, name="cos_c").ap()
    sin_c = nc.inline_tensor(sin_np, name="sin_c").ap()
    maskp_c = nc.inline_tensor(maskp_np, name="maskp_c").ap()
    ident_c = nc.inline_tensor(np.eye(128, dtype=np.float32), name="ident_c").ap()
    ones_kb_c = nc.inline_tensor(
        np.ones((128, 1), dtype=ml_dtypes.bfloat16), name="ones_kb_c"
    ).ap()
    ones_rf_c = nc.inline_tensor(
        np.ones((1, 128), dtype=np.float32), name="ones_rf_c"
    ).ap()

    with tile.TileContext(nc) as tc:
        with (
            tc.tile_pool(name="constp", bufs=1) as constp,
            tc.tile_pool(name="kvp", bufs=1) as kvp,
            tc.tile_pool(name="qstp", bufs=8) as qstp,
            tc.tile_pool(name="cspool", bufs=2) as cspool,
            tc.tile_pool(name="ropet", bufs=3) as ropet,
            tc.tile_pool(name="vtst", bufs=2) as vtst,
            tc.tile_pool(name="ppool", bufs=3) as ppool,
            tc.tile_pool(name="accp", bufs=2) as accp,
            tc.tile_pool(name="rsb", bufs=2) as rsb,
            tc.tile_pool(name="obf", bufs=2) as obf,
            tc.tile_pool(name="dramp", bufs=1, space="DRAM") as dramp,
            tc.tile_pool(name="pspp", bufs=1, space="PSUM") as pspp,
            tc.tile_pool(name="pop", bufs=1, space="PSUM") as pop,
        ):
            masksb = constp.tile([128, 4 * 512], BF16, tag="masksb")
            nc.gpsimd.dma_start(masksb[:], maskp_c)
            ident = constp.tile([128, 128], F32, tag="ident")
            nc.gpsimd.dma_start(ident[:], ident_c)
            ones_kb = constp.tile([128, 1], BF16, tag="ones_kb")
            nc.gpsimd.dma_start(ones_kb[:], ones_kb_c)
            ones_rf = constp.tile([1, 128], F32, tag="ones_rf")
            nc.gpsimd.dma_start(ones_rf[:], ones_rf_c)
            bias_t = constp.tile([128, 1], F32, tag="bias_t")
            nc.vector.memset(bias_t[:], EXP_BIAS)

            # per-batch K (rotated, [dh, l], bf16) and Vn ([key, dh], bf16)
            K2 = [kvp.tile([128, l], BF16, tag=f"K{i}", name=f"K{i}") for i in range(2)]
            Vn2 = [
                kvp.tile([128, l], BF16, tag=f"Vn{i}", name=f"Vn{i}")
                for i in range(2)
            ]

            # one bounce/gather pair per batch: rows = local heads * dh
            bounce = [
                dramp.tile([hpc * dh, l], BF16, tag=f"bounce{bb}",
                           name=f"bounce{bb}")
                for bb in range(b)
            ]
            gathered = [
                dramp.tile(
                    [n_cores * hpc * dh, l], BF16,
                    addr_space="Shared" if n_cores > 4 else "Local",
                    tag=f"gath{bb}", name=f"gath{bb}"
                )
                for bb in range(b)
            ]

            wq_r = wqT.rearrange("(k p) m -> p k m", p=128)
            wk_r = wkT.rearrange("(k p) m -> p k m", p=128)
            wv_r = wvT.rearrange("(k p) m -> p k m", p=128)
            xT_r = xT.rearrange("(k p) t -> p k t", p=128)

            # ---- attention micro-unit machinery --------------------------
            pending = []  # FIFO of (batch, closure)

            def drain(k):
                for _ in range(min(k, len(pending))):
                    pending.pop(0)[1]()

            def drain_all():
                while pending:
                    pending.pop(0)[1]()

            def drain_batch(bb):
                while pending and pending[0][0] == bb:
                    pending.pop(0)[1]()

            def make_group_units(bb, h, g, qt):
                """Attention for (batch bb, local head h, 512-query group g).
                qt: rotated q tile [128, 512] bf16.  Appends units to pending."""
                nkt = 4 * g + 4
                K = K2[bb]
                Vn = Vn2[bb]
                po = {}
                acc = {}
                state = {}

                def mk_scores(kt):
                    def u():
                        psp = pspp.tile([128, 512], F32, tag="psp", name="psp")
                        state[("psp", kt)] = psp
                        nc.tensor.matmul(
                            psp[:],
                            K[:, kt * 128 : (kt + 1) * 128],
                            qt[:],
                            start=True,
                            stop=True,
                            skip_group_check=True,
                        )
                    return u

                def mk_softpv(kt):
                    def u():
                        psp = state.pop(("psp", kt))
                        P = ppool.tile([128, 512], BF16, tag="P", name="P")
                        nc.scalar.activation(
                            P[:], psp[:], AF.Exp, scale=scale, bias=bias_t[:]
                        )
                        j = kt - 4 * g
                        if j >= 0:
                            nc.vector.tensor_mul(
                                P[:], P[:], masksb[:, j * 512 : (j + 1) * 512]
                            )
                        if kt == 0:
                            po["t"] = pop.tile([128, 512], F32, tag="po",
                                               name="po")
                            acc["t"] = accp.tile([128, 512], BF16, tag="acc",
                                                 name="acc")
                        nc.tensor.matmul(
                            po["t"][:],
                            Vn[:, kt * 128 : (kt + 1) * 128],
                            P[:],
                            start=(kt == 0),
                            stop=(kt == nkt - 1),
                            skip_group_check=True,
                        )
                        if kt == 0:
                            nc.vector.tensor_copy(acc["t"][:], P[:])
                        else:
                            nc.vector.tensor_add(acc["t"][:], acc["t"][:], P[:])
                        if kt == nkt - 1:
                            # evacuate po so its bank frees before the tail
                            po["sb"] = obf.tile([128, 512], F32, tag="posb",
                                                name="posb")
                            nc.scalar.activation(po["sb"][:], po["t"][:],
                                                 AF.Copy)
                    return u

                def tail():
                    # denominator sum + fast reciprocal + broadcast; the
                    # two matmuls borrow the score PSUM slot
                    pd_t = pspp.tile([1, 512], F32, tag="psp", name="pd")
                    nc.tensor.matmul(
                        pd_t[:], ones_kb[:], acc["t"][:],
                        start=True, stop=True, skip_group_check=True,
                    )
                    r_sb = rsb.tile([1, 512], F32, tag="r", name="r")
                    nc.vector.tensor_copy(r_sb[:], pd_t[:])
                    rr = rsb.tile([1, 512], F32, tag="rr", name="rr")
                    nc.vector.reciprocal_approx_fast(rr[:], r_sb[:])
                    pb_t = pspp.tile([128, 512], F32, tag="psp", name="pb")
                    nc.tensor.matmul(
                        pb_t[:], ones_rf[:], rr[:],
                        start=True, stop=True, skip_group_check=True,
                    )
                    ob = obf.tile([128, 512], BF16, tag="ob", name="ob")
                    nc.vector.tensor_mul(ob[:], po["sb"][:], pb_t[:])
                    nc.sync.dma_start(
                        bounce[bb][h * dh : (h + 1) * dh,
                                   g * 512 : (g + 1) * 512],
                        ob[:],
                    )

                # scores run one key-tile ahead of softmax+PV so exp latency
                # is always covered by in-flight PE work
                units = [mk_scores(0)]
                for kt in range(1, nkt):
                    units.append(mk_scores(kt))
                    units.append(mk_softpv(kt - 1))
                units.append(mk_softpv(nkt - 1))
                units.append(tail)
                pending.extend((bb, u) for u in units)

            # ---- fused projection + attention region ---------------------
            with (
                tc.tile_pool(name="wpool", bufs=1) as wpool,
                tc.tile_pool(name="xpool", bufs=3) as xpool,
                tc.tile_pool(name="psq", bufs=1, space="PSUM") as psq,
            ):
                wq_sb = wpool.tile([128, kt_d, hpc * dh], F32R, tag="wq")
                wk_sb = wpool.tile([128, kt_d, dh], F32R, tag="wk")
                wv_sb = wpool.tile([128, kt_d, dh], F32R, tag="wv")

                def emit_tg(tg):
                    bb = tg // 4
                    g = tg % 4
                    toff = tg * 512
                    tloc = g * 512

                    cos_sb = cspool.tile([128, 512], F32, tag="cos")
                    nc.gpsimd.dma_start(cos_sb[:], cos_c[:, toff : toff + 512])
                    sin_sb = cspool.tile([128, 512], F32, tag="sin")
                    nc.gpsimd.dma_start(sin_sb[:], sin_c[:, toff : toff + 512])

                    pq = [
                        psq.tile([128, 512], F32, tag=f"pq{o}", name=f"pq{o}")
                        for o in range(hpc)
                    ]
                    pk = psq.tile([128, 512], F32, tag="pk")
                    pv = psq.tile([128, 512], F32, tag="pv")
                    blocks = (
                        [(pk, wk_sb, 0), (pv, wv_sb, 0)]
                        + [(pq[o], wq_sb, o * dh) for o in range(hpc)]
                    )
                    for sub in range(nsub):
                        ks = slice(sub * ksub, (sub + 1) * ksub)
                        if tg == 0:
                            nc.gpsimd.dma_start(wq_sb[:, ks, :], wq_r[:, ks, :])
                            nc.gpsimd.dma_start(wk_sb[:, ks, :], wk_r[:, ks, :])
                            nc.gpsimd.dma_start(wv_sb[:, ks, :], wv_r[:, ks, :])
                        xs = xpool.tile([128, ksub, 512], F32R, tag="xs")
                        nc.sync.dma_start(xs[:], xT_r[:, ks, toff : toff + 512])
                        for dst, w_sb, o0 in blocks:
                            for k in range(ksub):
                                kt = sub * ksub + k
                                nc.tensor.matmul(
                                    dst[:], w_sb[:, kt, o0 : o0 + dh],
                                    xs[:, k, :],
                                    start=(kt == 0), stop=(kt == kt_d - 1),
                                )
                            drain(2)

                    def rope(dst_ap, src_ap):
                        # dst[0:64]  = s[0:64]*cos - s[64:]*sin
                        # dst[64:]   = s[64:]*cos + s[0:64]*sin  (out bf16)
                        tc_ = ropet.tile([128, 512], F32, tag="rtc", name="rtc")
                        ts_ = ropet.tile([128, 512], F32, tag="rts", name="rts")
                        nc.vector.tensor_mul(tc_[:], src_ap, cos_sb[:])
                        nc.vector.tensor_mul(
                            ts_[0:64, :], src_ap[64:128, :], sin_sb[64:128, :]
                        )
                        nc.vector.tensor_mul(
                            ts_[64:128, :], src_ap[0:64, :], sin_sb[0:64, :]
                        )
                        nc.vector.tensor_sub(
                            dst_ap[0:64, :], tc_[0:64, :], ts_[0:64, :]
                        )
                        nc.vector.tensor_add(
                            dst_ap[64:128, :], tc_[64:128, :], ts_[64:128, :]
                        )

                    # k: rope from PSUM into K2 (bf16)
                    rope(K2[bb][:, tloc : tloc + 512], pk[:])
                    # v: ACT copy + PE transpose into Vn (transposes borrow
                    # the score PSUM slot)
                    vt = vtst.tile([128, 512], F32, tag="vt", name="vt")
                    nc.scalar.activation(vt[:], pv[:], AF.Copy)
                    pt = pspp.tile([128, 512], F32, tag="psp", name="pt")
                    for j in range(4):
                        nc.tensor.transpose(
                            pt[:, j * 128 : (j + 1) * 128],
                            vt[:, j * 128 : (j + 1) * 128],
                            ident[:],
                        )
                    nc.vector.tensor_copy(Vn2[bb][:, tloc : tloc + 512], pt[:])

                    # q ropes (bf16) + enqueue this token group's attention
                    qts = [qstp.tile([128, 512], BF16, tag="qst", name="qst")
                           for _ in range(hpc)]
                    for o in range(hpc):
                        rope(qts[o][:], pq[o][:])
                        make_group_units(bb, o, g, qts[o])
                        drain(2)

                for tg in range(tg_n):
                    emit_tg(tg)
                    if tg == 5:
                        # batch-0 groups all enqueued by tg3; force
                        # stragglers so bounce writes precede the collective
                        drain_batch(0)
                        nc.gpsimd.collective_compute(
                            "AllGather",
                            mybir.AluOpType.bypass,
                            replica_groups=[list(range(n_cores))],
                            ins=[bounce[0].opt()],
                            outs=[gathered[0].opt()],
                        )

            # wq/wk/wv/xs pools and proj PSUM released here.
            with (
                tc.tile_pool(name="wopool", bufs=1) as wopool,
                tc.tile_pool(name="ogpool", bufs=2) as ogpool,
                tc.tile_pool(name="outst", bufs=3) as outst,
                tc.tile_pool(name="pso", bufs=2, space="PSUM") as pso,
            ):
                wo_sb = wopool.tile([128, kt_d, mpc], BF16, tag="wo")
                nc.gpsimd.dma_start(
                    wo_sb[:], woT.rearrange("(k p) m -> p k m", p=128)
                )

                def oproj_slab(bb, tgl):
                    # gathered rows: rank r block holds global heads
                    # 4r..4r+3, so k-tile index == Wo column block index
                    g_r = gathered[bb][:].rearrange("(k p) t -> p k t", p=128)
                    og = ogpool.tile([128, kt_d, 512], BF16, tag="og", name="og")
                    nc.gpsimd.dma_start(
                        og[:], g_r[:, :, tgl * 512 : (tgl + 1) * 512]
                    )
                    # keep leftover attention units ahead of matmuls that
                    # wait on the og/wo DMAs
                    drain(2)
                    for m in range(mpc // 128):
                        pp = pso.tile([128, 512], F32, tag="pp", name="pp")
                        for kt in range(kt_d):
                            nc.tensor.matmul(
                                pp[:],
                                wo_sb[:, kt, m * 128 : (m + 1) * 128],
                                og[:, kt, :],
                                start=(kt == 0),
                                stop=(kt == kt_d - 1),
                            )
                            if kt % 4 == 3:
                                drain(1)
                        ot = outst.tile([128, 512], F32, tag="ot", name="ot")
                        nc.scalar.activation(ot[:], pp[:], AF.Copy)
                        nc.sync.dma_start(
                            outT[
                                m * 128 : (m + 1) * 128,
                                bb * l + tgl * 512 : bb * l + (tgl + 1) * 512,
                            ],
                            ot[:],
                        )

                # batch-0 o_proj immediately (its gather landed mid-kernel);
                # leftover batch-1 attention drains under it, then the
                # batch-1 AllGather fires and hides under the rest
                drain(6)
                oproj_slab(0, 0)
                oproj_slab(0, 1)
                drain_all()
                nc.gpsimd.collective_compute(
                    "AllGather",
                    mybir.AluOpType.bypass,
                    replica_groups=[list(range(n_cores))],
                    ins=[bounce[1].opt()],
                    outs=[gathered[1].opt()],
                )
                oproj_slab(0, 2)
                oproj_slab(0, 3)
                for tgl in range(l // 512):
                    oproj_slab(1, tgl)

    nc.compile()
    return nc


_NC_CACHE = {}


def _get_nc(key=(N_CORES, B, L, N_HEADS, N_KV)):
    if key not in _NC_CACHE:
        _NC_CACHE[key] = _build(*key)
    return _NC_CACHE[key]


def make_in_maps(x, Wq, Wk, Wv, Wo, n_cores=N_CORES):
    import ml_dtypes

    b, l, d = x.shape
    nh = Wq.shape[0] // HEAD_DIM
    hpc = nh // n_cores
    mpc = d // n_cores
    xT = np.ascontiguousarray(x.reshape(b * l, d).T.astype(np.float32))
    in_maps = []
    for c in range(n_cores):
        wq_c = np.ascontiguousarray(
            Wq[c * hpc * HEAD_DIM : (c + 1) * hpc * HEAD_DIM, :].T.astype(np.float32)
        )
        wk_c = np.ascontiguousarray(
            Wk[c * HEAD_DIM : (c + 1) * HEAD_DIM, :].T.astype(np.float32)
        )
        wv_c = np.ascontiguousarray(
            Wv[c * HEAD_DIM : (c + 1) * HEAD_DIM, :].T.astype(np.float32)
        )
        wo_c = np.ascontiguousarray(
            Wo[c * mpc : (c + 1) * mpc, :].T.astype(ml_dtypes.bfloat16)
        )
        in_maps.append(
            {"xT": xT, "wqT": wq_c, "wkT": wk_c, "wvT": wv_c, "woT": wo_c}
        )
    return in_maps


def assemble_out(results, b, l, d):
    parts = [r["outT"] for r in results]
    outT = np.concatenate(parts, axis=0)  # [D, T]
    return np.ascontiguousarray(outT.T).reshape(b, l, d).astype(np.float32)


def kernel(x, Wq, Wk, Wv, Wo, trace=False, tmpdir=None):
    x = np.asarray(x, dtype=np.float32)
    nc = _get_nc()
    in_maps = make_in_maps(x, Wq, Wk, Wv, Wo)
    res = run_bass_kernel_spmd(
        nc, in_maps, list(range(N_CORES)), trace=trace, tmpdir=tmpdir
    )
    out = assemble_out(res.results, *x.shape)
    if trace:
        return out, res
    return out


if __name__ == "__main__":
    rng = np.random.default_rng(0)
    s = 0.02
    x = rng.standard_normal((B, L, D)).astype(np.float32)
    Wq = (rng.standard_normal((D, D)) * s).astype(np.float32)
    Wk = (rng.standard_normal((N_KV * HEAD_DIM, D)) * s).astype(np.float32)
    Wv = (rng.standard_normal((N_KV * HEAD_DIM, D)) * s).astype(np.float32)
    Wo = (rng.standard_normal((D, D)) * s).astype(np.float32)
    out = kernel(x, Wq, Wk, Wv, Wo)
    print(out.shape, out.dtype)


# revision 24
# speedup vs baseline: 1.3516x; 1.0711x over previous
"""Trainium2 Bass kernel for nn_Attention_43963285242601.

GQA attention block: q/k/v projections + RoPE + causal attention + o_proj,
tensor-parallel over 8 NeuronCores.

Sharding (core c of 8):
  - q-heads 4c..4c+3 and kv-head c: Wq/Wk/Wv column (head) shards,
    attention fully local per head group.
  - o_proj sharded over Wo ROWS (output features): every core computes
    out[:, 512c:512c+512] and needs the full attention output, distributed
    via one AllGather per batch (bf16).  The host concatenates the 8
    feature shards: no all-reduce needed.

v6: fused emission stream.  Projections (six PSUM accumulators, sub-major
like the baseline) run per 512-token group; attention for earlier groups
is emitted as single-key-tile micro-units interleaved between projection
matmul blocks, so softmax (ACT) latency and all softmax-tail latencies
hide under PE work that does not depend on them.  K / q / scores / P run
in bf16 (bf16 LDWEIGHTS fully hides under the matmul stream; fp32
weights do not), the causal mask is a post-exp bf16 multiply on DVE, the
PV accumulator is evacuated to SBUF immediately (its PSUM bank frees
after one ACT copy), and the softmax denominator uses sum/broadcast
matmuls that borrow the score PSUM slot plus a fast-approximate
reciprocal (the exact DVE reciprocal costs 4us and serialized the DVE
queue).  One AllGather per batch: batch 0 fires mid-kernel, batch 1
hides under o_proj(b0); o_proj uses an identity head mapping thanks to
the combined gather layout.
"""

import numpy as np

import concourse.bacc as bacc
import concourse.mybir as mybir
import concourse.tile as tile
from concourse.bass_utils import run_bass_kernel_spmd

F32 = mybir.dt.float32
F32R = mybir.dt.float32r
BF16 = mybir.dt.bfloat16
AF = mybir.ActivationFunctionType

N_CORES = 8
B, L = 2, 2048
N_HEADS, N_KV = 32, 8
HEAD_DIM = 128
D = N_HEADS * HEAD_DIM
THETA = 500000.0

EXP_BIAS = -8.0


def _rope_tables(t_all, l, dh):
    half = dh // 2
    inv = 1.0 / (THETA ** (np.arange(half, dtype=np.float64) * 2.0 / dh))
    pos = np.arange(t_all, dtype=np.float64) % l
    ang = inv[:, None] * pos[None, :]  # [half, T]
    cos = np.cos(ang)
    sin = np.sin(ang)
    return (
        np.concatenate([cos, cos], 0).astype(np.float32),
        np.concatenate([sin, sin], 0).astype(np.float32),
    )


def _build(n_cores=N_CORES, b=B, l=L, nh=N_HEADS, nkv=N_KV):
    import ml_dtypes

    dh = HEAD_DIM
    d = nh * dh
    t_all = b * l
    hpc = nh // n_cores  # q heads per core
    assert nkv == n_cores, "one kv head per core"
    mpc = d // n_cores  # o_proj output features per core
    kt_d = d // dh  # contraction tiles for projections
    tg_n = t_all // 512  # 512-wide token groups
    ksub = 4  # k-tiles per x subslab load
    nsub = kt_d // ksub
    scale = dh ** -0.5

    nc = bacc.Bacc(
        "TRN2", target_bir_lowering=False, debug=False, num_devices=n_cores
    )

    xT = nc.dram_tensor("xT", [d, t_all], F32R, kind="ExternalInput").ap()
    wqT = nc.dram_tensor("wqT", [d, hpc * dh], F32R, kind="ExternalInput").ap()
    wkT = nc.dram_tensor("wkT", [d, dh], F32R, kind="ExternalInput").ap()
    wvT = nc.dram_tensor("wvT", [d, dh], F32R, kind="ExternalInput").ap()
    woT = nc.dram_tensor("woT", [d, mpc], BF16, kind="ExternalInput").ap()
    outT = nc.dram_tensor("outT", [mpc, t_all], F32, kind="ExternalOutput").ap()

    # compile-time constants
    cos_np, sin_np = _rope_tables(t_all, l, dh)
    # multiplicative bf16 causal mask for the 4 diagonal key tiles of each
    # 512-query group: maskP[k, j*512 + q] = (128*j + k <= q)
    k_idx = np.arange(128)[:, None]
    q_idx = np.arange(512)[None, :]
    maskp_np = np.zeros((128, 4 * 512), dtype=ml_dtypes.bfloat16)
    for j in range(4):
        maskp_np[:, j * 512 : (j + 1) * 512] = (128 * j + k_idx <= q_idx).astype(
            ml_dtypes.bfloat16
        )
    cos_c = nc.inline_tensor(cos_np, name="cos_c").ap()
    sin_c = nc.inline_tensor(sin_np, name="sin_c").ap()
    maskp_c = nc.inline_tensor(maskp_np, name="maskp_c").ap()
    ident_c = nc.inline_tensor(np.eye(128, dtype=np.float32), name="ident_c").ap()
    ones_kb_c = nc.inline_tensor(
        np.ones((128, 1), dtype=ml_dtypes.bfloat16), name="ones_kb_c"
    ).ap()
    ones_rf_c = nc.inline_tensor(
        np.ones((1, 128), dtype=np.float32), name="ones_rf_c"
    ).ap()

    with tile.TileContext(nc) as tc:
        with (
            tc.tile_pool(name="constp", bufs=1) as constp,
            tc.tile_pool(name="kvp", bufs=1) as kvp,
            tc.tile_pool(name="qstp", bufs=8) as qstp,
            tc.tile_pool(name="cspool", bufs=2) as cspool,
            tc.tile_pool(name="ropet", bufs=3) as ropet,
            tc.tile_pool(name="vtst", bufs=2) as vtst,
            tc.tile_pool(name="ppool", bufs=3) as ppool,
            tc.tile_pool(name="accp", bufs=2) as accp,
            tc.tile_pool(name="rsb", bufs=2) as rsb,
            tc.tile_pool(name="obf", bufs=2) as obf,
            tc.tile_pool(name="dramp", bufs=1, space="DRAM") as dramp,
            tc.tile_pool(name="pspp", bufs=1, space="PSUM") as pspp,
            tc.tile_pool(name="pop", bufs=1, space="PSUM") as pop,
        ):
            masksb = constp.tile([128, 4 * 512], BF16, tag="masksb")
            nc.gpsimd.dma_start(masksb[:], maskp_c)
            ident = constp.tile([128, 128], F32, tag="ident")
            nc.gpsimd.dma_start(ident[:], ident_c)
            ones_kb = constp.tile([128, 1], BF16, tag="ones_kb")
            nc.gpsimd.dma_start(ones_kb[:], ones_kb_c)
            ones_rf = constp.tile([1, 128], F32, tag="ones_rf")
            nc.gpsimd.dma_start(ones_rf[:], ones_rf_c)
            bias_t = constp.tile([128, 1], F32, tag="bias_t")
            nc.vector.memset(bias_t[:], EXP_BIAS)

            # per-batch K (rotated, [dh, l], bf16) and Vn ([key, dh], bf16)
            K2 = [kvp.tile([128, l], BF16, tag=f"K{i}", name=f"K{i}") for i in range(2)]
            Vn2 = [
                kvp.tile([128, l], BF16, tag=f"Vn{i}", name=f"Vn{i}")
                for i in range(2)
            ]

            # one bounce/gather pair per batch: rows = local heads * dh
            bounce = [
                dramp.tile([hpc * dh, l], BF16, tag=f"bounce{bb}",
                           name=f"bounce{bb}")
                for bb in range(b)
            ]
            gathered = [
                dramp.tile(
                    [n_cores * hpc * dh, l], BF16,
                    addr_space="Shared" if n_cores > 4 else "Local",
                    tag=f"gath{bb}", name=f"gath{bb}"
                )
                for bb in range(b)
            ]

            wq_r = wqT.rearrange("(k p) m -> p k m", p=128)
            wk_r = wkT.rearrange("(k p) m -> p k m", p=128)
            wv_r = wvT.rearrange("(k p) m -> p k m", p=128)
            xT_r = xT.rearrange("(k p) t -> p k t", p=128)

            # ---- attention micro-unit machinery --------------------------
            pending = []  # FIFO of (batch, closure)

            def drain(k):
                for _ in range(min(k, len(pending))):
                    pending.pop(0)[1]()

            def drain_all():
                while pending:
                    pending.pop(0)[1]()

            def drain_batch(bb):
                while pending and pending[0][0] == bb:
                    pending.pop(0)[1]()

            def make_group_units(bb, h, g, qt):
                """Attention for (batch bb, local head h, 512-query group g).
                qt: rotated q tile [128, 512] bf16.  Appends units to pending."""
                nkt = 4 * g + 4
                K = K2[bb]
                Vn = Vn2[bb]
                po = {}
                acc = {}
                state = {}

                def mk_scores(kt):
                    def u():
                        psp = pspp.tile([128, 512], F32, tag="psp", name="psp")
                        state[("psp", kt)] = psp
                        nc.tensor.matmul(
                            psp[:],
                            K[:, kt * 128 : (kt + 1) * 128],
                            qt[:],
                            start=True,
                            stop=True,
                            skip_group_check=True,
                        )
                    return u

                def mk_softpv(kt):
                    def u():
                        psp = state.pop(("psp", kt))
                        P = ppool.tile([128, 512], BF16, tag="P", name="P")
                        nc.scalar.activation(
                            P[:], psp[:], AF.Exp, scale=scale, bias=bias_t[:]
                        )
                        j = kt - 4 * g
                        if j >= 0:
                            nc.vector.tensor_mul(
                                P[:], P[:], masksb[:, j * 512 : (j + 1) * 512]
                            )
                        if kt == 0:
                            po["t"] = pop.tile([128, 512], F32, tag="po",
                                               name="po")
                            acc["t"] = accp.tile([128, 512], BF16, tag="acc",
                                                 name="acc")
                        nc.tensor.matmul(
                            po["t"][:],
                            Vn[:, kt * 128 : (kt + 1) * 128],
                            P[:],
                            start=(kt == 0),
                            stop=(kt == nkt - 1),
                            skip_group_check=True,
                        )
                        if kt == 0:
                            nc.vector.tensor_copy(acc["t"][:], P[:])
                        else:
                            nc.vector.tensor_add(acc["t"][:], acc["t"][:], P[:])
                        if kt == nkt - 1:
                            # evacuate po so its bank frees before the tail
                            po["sb"] = obf.tile([128, 512], F32, tag="posb",
                                                name="posb")
                            nc.scalar.activation(po["sb"][:], po["t"][:],
                                                 AF.Copy)
                    return u

                def tail():
                    # denominator sum + fast reciprocal + broadcast; the
                    # two matmuls borrow the score PSUM slot
                    pd_t = pop.tile([1, 512], F32, tag="po", name="pd")
                    nc.tensor.matmul(
                        pd_t[:], ones_kb[:], acc["t"][:],
                        start=True, stop=True, skip_group_check=True,
                    )
                    r_sb = rsb.tile([1, 512], F32, tag="r", name="r")
                    nc.vector.tensor_copy(r_sb[:], pd_t[:])
                    rr = rsb.tile([1, 512], F32, tag="rr", name="rr")
                    nc.vector.reciprocal_approx_fast(rr[:], r_sb[:])
                    pb_t = pop.tile([128, 512], F32, tag="po", name="pb")
                    nc.tensor.matmul(
                        pb_t[:], ones_rf[:], rr[:],
                        start=True, stop=True, skip_group_check=True,
                    )
                    ob = obf.tile([128, 512], BF16, tag="ob", name="ob")
                    nc.vector.tensor_mul(ob[:], po["sb"][:], pb_t[:])
                    nc.sync.dma_start(
                        bounce[bb][h * dh : (h + 1) * dh,
                                   g * 512 : (g + 1) * 512],
                        ob[:],
                    )

                # scores run one key-tile ahead of softmax+PV so exp latency
                # is always covered by in-flight PE work
                units = [mk_scores(0)]
                for kt in range(1, nkt):
                    units.append(mk_scores(kt))
                    units.append(mk_softpv(kt - 1))
                units.append(mk_softpv(nkt - 1))
                units.append(tail)
                pending.extend((bb, u) for u in units)

            # ---- fused projection + attention region ---------------------
            with (
                tc.tile_pool(name="wpool", bufs=1) as wpool,
                tc.tile_pool(name="xpool", bufs=4) as xpool,
                tc.tile_pool(name="psq", bufs=1, space="PSUM") as psq,
            ):
                wq_sb = wpool.tile([128, kt_d, hpc * dh], F32R, tag="wq")
                wk_sb = wpool.tile([128, kt_d, dh], F32R, tag="wk")
                wv_sb = wpool.tile([128, kt_d, dh], F32R, tag="wv")

                def emit_tg(tg):
                    bb = tg // 4
                    g = tg % 4
                    toff = tg * 512
                    tloc = g * 512

                    cos_sb = cspool.tile([128, 512], F32, tag="cos")
                    nc.gpsimd.dma_start(cos_sb[:], cos_c[:, toff : toff + 512])
                    sin_sb = cspool.tile([128, 512], F32, tag="sin")
                    nc.gpsimd.dma_start(sin_sb[:], sin_c[:, toff : toff + 512])

                    pq = [
                        psq.tile([128, 512], F32, tag=f"pq{o}", name=f"pq{o}")
                        for o in range(hpc)
                    ]
                    pk = psq.tile([128, 512], F32, tag="pk")
                    pv = psq.tile([128, 512], F32, tag="pv")
                    blocks = (
                        [(pk, wk_sb, 0), (pv, wv_sb, 0)]
                        + [(pq[o], wq_sb, o * dh) for o in range(hpc)]
                    )
                    for sub in range(nsub):
                        ks = slice(sub * ksub, (sub + 1) * ksub)
                        if tg == 0:
                            nc.gpsimd.dma_start(wq_sb[:, ks, :], wq_r[:, ks, :])
                            nc.gpsimd.dma_start(wk_sb[:, ks, :], wk_r[:, ks, :])
                            nc.gpsimd.dma_start(wv_sb[:, ks, :], wv_r[:, ks, :])
                        xs = xpool.tile([128, ksub, 512], F32R, tag="xs")
                        nc.sync.dma_start(xs[:], xT_r[:, ks, toff : toff + 512])
                        for dst, w_sb, o0 in blocks:
                            for k in range(ksub):
                                kt = sub * ksub + k
                                nc.tensor.matmul(
                                    dst[:], w_sb[:, kt, o0 : o0 + dh],
                                    xs[:, k, :],
                                    start=(kt == 0), stop=(kt == kt_d - 1),
                                )
                            drain(3 if g == 3 else 2)

                    def rope(dst_ap, src_ap):
                        # dst[0:64]  = s[0:64]*cos - s[64:]*sin
                        # dst[64:]   = s[64:]*cos + s[0:64]*sin  (out bf16)
                        tc_ = ropet.tile([128, 512], F32, tag="rtc", name="rtc")
                        ts_ = ropet.tile([128, 512], F32, tag="rts", name="rts")
                        nc.vector.tensor_mul(tc_[:], src_ap, cos_sb[:])
                        nc.vector.tensor_mul(
                            ts_[0:64, :], src_ap[64:128, :], sin_sb[64:128, :]
                        )
                        nc.vector.tensor_mul(
                            ts_[64:128, :], src_ap[0:64, :], sin_sb[0:64, :]
                        )
                        nc.vector.tensor_sub(
                            dst_ap[0:64, :], tc_[0:64, :], ts_[0:64, :]
                        )
                        nc.vector.tensor_add(
                            dst_ap[64:128, :], tc_[64:128, :], ts_[64:128, :]
                        )

                    # k: rope from PSUM into K2 (bf16)
                    rope(K2[bb][:, tloc : tloc + 512], pk[:])
                    # v: ACT copy + PE transpose into Vn (transposes borrow
                    # the score PSUM slot)
                    vt = vtst.tile([128, 512], F32, tag="vt", name="vt")
                    nc.scalar.activation(vt[:], pv[:], AF.Copy)
                    pt = pspp.tile([128, 512], F32, tag="psp", name="pt")
                    for j in range(4):
                        nc.tensor.transpose(
                            pt[:, j * 128 : (j + 1) * 128],
                            vt[:, j * 128 : (j + 1) * 128],
                            ident[:],
                        )
                    nc.vector.tensor_copy(Vn2[bb][:, tloc : tloc + 512], pt[:])

                    # q ropes (bf16) + enqueue this token group's attention
                    qts = [qstp.tile([128, 512], BF16, tag="qst", name="qst")
                           for _ in range(hpc)]
                    for o in range(hpc):
                        rope(qts[o][:], pq[o][:])
                        make_group_units(bb, o, g, qts[o])
                        drain(2)

                for tg in range(tg_n):
                    emit_tg(tg)
                    if tg == 5:
                        # batch-0 groups all enqueued by tg3; force
                        # stragglers so bounce writes precede the collective
                        drain_batch(0)
                        nc.gpsimd.collective_compute(
                            "AllGather",
                            mybir.AluOpType.bypass,
                            replica_groups=[list(range(n_cores))],
                            ins=[bounce[0].opt()],
                            outs=[gathered[0].opt()],
                        )

            # wq/wk/wv/xs pools and proj PSUM released here.
            with (
                tc.tile_pool(name="wopool", bufs=1) as wopool,
                tc.tile_pool(name="ogpool", bufs=2) as ogpool,
                tc.tile_pool(name="outst", bufs=3) as outst,
                tc.tile_pool(name="pso", bufs=2, space="PSUM") as pso,
            ):
                wo_sb = wopool.tile([128, kt_d, mpc], BF16, tag="wo")
                nc.gpsimd.dma_start(
                    wo_sb[:], woT.rearrange("(k p) m -> p k m", p=128)
                )

                def oproj_slab(bb, tgl):
                    # gathered rows: rank r block holds global heads
                    # 4r..4r+3, so k-tile index == Wo column block index
                    g_r = gathered[bb][:].rearrange("(k p) t -> p k t", p=128)
                    og = ogpool.tile([128, kt_d, 512], BF16, tag="og", name="og")
                    nc.gpsimd.dma_start(
                        og[:], g_r[:, :, tgl * 512 : (tgl + 1) * 512]
                    )
                    # keep leftover attention units ahead of matmuls that
                    # wait on the og/wo DMAs
                    drain(2)
                    for m in range(mpc // 128):
                        pp = pso.tile([128, 512], F32, tag="pp", name="pp")
                        for kt in range(kt_d):
                            nc.tensor.matmul(
                                pp[:],
                                wo_sb[:, kt, m * 128 : (m + 1) * 128],
                                og[:, kt, :],
                                start=(kt == 0),
                                stop=(kt == kt_d - 1),
                            )
                            if kt % 4 == 3:
                                drain(1)
                        ot = outst.tile([128, 512], F32, tag="ot", name="ot")
                        nc.scalar.activation(ot[:], pp[:], AF.Copy)
                        nc.sync.dma_start(
                            outT[
                                m * 128 : (m + 1) * 128,
                                bb * l + tgl * 512 : bb * l + (tgl + 1) * 512,
                            ],
                            ot[:],
                        )

                # the g==3 drain boost leaves few batch-1 stragglers:
                # flush them, fire AllGather(b1) immediately, and hide it
                # under o_proj(b0)
                drain_all()
                nc.gpsimd.collective_compute(
                    "AllGather",
                    mybir.AluOpType.bypass,
                    replica_groups=[list(range(n_cores))],
                    ins=[bounce[1].opt()],
                    outs=[gathered[1].opt()],
                )
                for bb in range(b):
                    for tgl in range(l // 512):
                        oproj_slab(bb, tgl)

    nc.compile()
    return nc


_NC_CACHE = {}


def _get_nc(key=(N_CORES, B, L, N_HEADS, N_KV)):
    if key not in _NC_CACHE:
        _NC_CACHE[key] = _build(*key)
    return _NC_CACHE[key]


def make_in_maps(x, Wq, Wk, Wv, Wo, n_cores=N_CORES):
    import ml_dtypes

    b, l, d = x.shape
    nh = Wq.shape[0] // HEAD_DIM
    hpc = nh // n_cores
    mpc = d // n_cores
    xT = np.ascontiguousarray(x.reshape(b * l, d).T.astype(np.float32))
    in_maps = []
    for c in range(n_cores):
        wq_c = np.ascontiguousarray(
            Wq[c * hpc * HEAD_DIM : (c + 1) * hpc * HEAD_DIM, :].T.astype(np.float32)
        )
        wk_c = np.ascontiguousarray(
            Wk[c * HEAD_DIM : (c + 1) * HEAD_DIM, :].T.astype(np.float32)
        )
        wv_c = np.ascontiguousarray(
            Wv[c * HEAD_DIM : (c + 1) * HEAD_DIM, :].T.astype(np.float32)
        )
        wo_c = np.ascontiguousarray(
            Wo[c * mpc : (c + 1) * mpc, :].T.astype(ml_dtypes.bfloat16)
        )
        in_maps.append(
            {"xT": xT, "wqT": wq_c, "wkT": wk_c, "wvT": wv_c, "woT": wo_c}
        )
    return in_maps


def assemble_out(results, b, l, d):
    parts = [r["outT"] for r in results]
    outT = np.concatenate(parts, axis=0)  # [D, T]
    return np.ascontiguousarray(outT.T).reshape(b, l, d).astype(np.float32)


def kernel(x, Wq, Wk, Wv, Wo, trace=False, tmpdir=None):
    x = np.asarray(x, dtype=np.float32)
    nc = _get_nc()
    in_maps = make_in_maps(x, Wq, Wk, Wv, Wo)
    res = run_bass_kernel_spmd(
        nc, in_maps, list(range(N_CORES)), trace=trace, tmpdir=tmpdir
    )
    out = assemble_out(res.results, *x.shape)
    if trace:
        return out, res
    return out


if __name__ == "__main__":
    rng = np.random.default_rng(0)
    s = 0.02
    x = rng.standard_normal((B, L, D)).astype(np.float32)
    Wq = (rng.standard_normal((D, D)) * s).astype(np.float32)
    Wk = (rng.standard_normal((N_KV * HEAD_DIM, D)) * s).astype(np.float32)
    Wv = (rng.standard_normal((N_KV * HEAD_DIM, D)) * s).astype(np.float32)
    Wo = (rng.standard_normal((D, D)) * s).astype(np.float32)
    out = kernel(x, Wq, Wk, Wv, Wo)
    print(out.shape, out.dtype)
